# revision 1
# baseline (speedup 1.0000x reference)
"""Gemma4 sliding-window attention, tensor-parallel over 8 NeuronCores.

Sharding (per spec hint): one Q head per core (HQ=8). Each core projects
its own q head + the matching GQA kv head (h//2), applies RMSNorm + RoPE,
runs banded sliding-window attention (WIN=1024 -> each 1024-row query
block only attends to a 2048-wide key band), computes the partial o_proj
contribution for its head slice of Wo, and the partials are all-reduced.
"""

import numpy as np
import jax
import jax.numpy as jnp
from functools import partial

B, S, H = 1, 4096, 2048
HQ, HKV, D = 8, 4, 256
WIN = 1024
SOFTCAP = 50.0
EPS = 1e-6
NB = S // WIN  # query blocks

_cache = {}


def _rms(x, w=None):
    ms = jnp.mean(x * x, axis=-1, keepdims=True) + EPS
    y = x * jax.lax.rsqrt(ms)
    return y * w if w is not None else y


def _rope(x, cos, sin):
    x1, x2 = jnp.split(x, 2, axis=-1)
    rot = jnp.concatenate([-x2, x1], axis=-1)
    return x * cos + rot * sin


def _banded_head(wq, wk, wv, wo, qw, kw, hs, cos, sin, mask_ext):
    # wq/wk/wv: [D,H]; wo: [H,D]; hs: [S,H]; cos/sin: [S,D]
    # mask_ext: [NB, WIN, 2*WIN] additive mask per query block over its key band
    q = _rope(_rms(hs @ wq.T, qw), cos, sin)          # [S,D]
    k = _rope(_rms(hs @ wk.T, kw), cos, sin)          # [S,D]
    v = _rms(hs @ wv.T)                               # [S,D]

    pad = jnp.zeros((WIN, D), dtype=hs.dtype)
    k_pad = jnp.concatenate([pad, k], axis=0)         # [S+WIN, D]
    v_pad = jnp.concatenate([pad, v], axis=0)
    k_ext = jnp.stack([k_pad[i * WIN:i * WIN + 2 * WIN] for i in range(NB)])
    v_ext = jnp.stack([v_pad[i * WIN:i * WIN + 2 * WIN] for i in range(NB)])
    qb = q.reshape(NB, WIN, D)

    s = jnp.einsum('bqd,bkd->bqk', qb, k_ext)         # [NB, WIN, 2WIN]
    s = jnp.tanh(s / SOFTCAP) * SOFTCAP + mask_ext
    a = jax.nn.softmax(s, axis=-1)
    ob = jnp.einsum('bqk,bkd->bqd', a, v_ext)         # [NB, WIN, D]
    out = ob.reshape(S, D)
    part = out @ wo.T                                 # [S, H] partial
    return jax.lax.psum(part, 'x')


def _full_head(wq, wk, wv, wo, qw, kw, hs, cos, sin, mask):
    q = _rope(_rms(hs @ wq.T, qw), cos, sin)
    k = _rope(_rms(hs @ wk.T, kw), cos, sin)
    v = _rms(hs @ wv.T)
    s = q @ k.T
    s = jnp.tanh(s / SOFTCAP) * SOFTCAP + mask
    a = jax.nn.softmax(s, axis=-1)
    part = (a @ v) @ wo.T
    return jax.lax.psum(part, 'x')


def _get_fn(banded):
    key = ('banded' if banded else 'full')
    if key not in _cache:
        fn = _banded_head if banded else _full_head
        _cache[key] = jax.pmap(
            fn, axis_name='x', devices=jax.devices()[:8],
            in_axes=(0, 0, 0, 0, None, None, None, None, None, None))
    return _cache[key]


def kernel(hidden_states, cos, sin, attention_mask, Wq, Wk, Wv, Wo,
           q_norm_w, k_norm_w):
    hs = np.asarray(hidden_states, dtype=np.float32)[0]       # [S,H]
    cos2 = np.asarray(cos, dtype=np.float32)[0]               # [S,D]
    sin2 = np.asarray(sin, dtype=np.float32)[0]
    mask = np.asarray(attention_mask, dtype=np.float32)[0, 0]  # [S,S]

    wq = np.asarray(Wq, dtype=np.float32).reshape(HQ, D, H)
    wk_all = np.asarray(Wk, dtype=np.float32).reshape(HKV, D, H)
    wv_all = np.asarray(Wv, dtype=np.float32).reshape(HKV, D, H)
    rep = np.arange(HQ) // (HQ // HKV)
    wk = wk_all[rep]                                          # [HQ,D,H]
    wv = wv_all[rep]
    wo = np.asarray(Wo, dtype=np.float32).reshape(H, HQ, D).transpose(1, 0, 2)
    qw = np.asarray(q_norm_w, dtype=np.float32)
    kw = np.asarray(k_norm_w, dtype=np.float32)

    # Banded fast path is only valid when the mask actually enforces the
    # sliding window; verify cheaply on host, else run full attention.
    r = S - 1
    banded = (mask[r, r - WIN] < -1e8) and (mask[0, S - 1] < -1e8)
    if banded:
        neg = np.float32(-1e9)
        m_pad = np.concatenate(
            [np.full((S, WIN), neg, dtype=np.float32), mask], axis=1)
        mask_ext = np.stack([
            m_pad[i * WIN:(i + 1) * WIN, i * WIN:i * WIN + 2 * WIN]
            for i in range(NB)])
        out = _get_fn(True)(wq, wk, wv, wo, qw, kw, hs, cos2, sin2, mask_ext)
    else:
        out = _get_fn(False)(wq, wk, wv, wo, qw, kw, hs, cos2, sin2, mask)

    return np.asarray(out[0], dtype=np.float32)[None]          # [1,S,H]
